# revision 45
# baseline (speedup 1.0000x reference)
"""Multi-head attention (b=16, n=512, d=768, h=12) on 8 trn2 NeuronCores.

Strategy: pure data-parallel over batch (2 batches per core), no collectives.
Host pre-transposes/casts the per-core x slice to xT bf16 [768, 1024] and
casts weights to bf16; all matmuls run bf16 with fp32 PSUM accumulation.

Per-core dataflow (P = 128 partitions):
  qkT[m]  = Wqkv[:, m-tile]^T @ xT          -> [outfeat, tok] (bf16, +bias)
  v_aug   = x @ Wv  stored per head as [v_h | ones64]  (natural [tok, feat])
  scoresT = k_h @ q_h^T   (2 heads row-packed on the PE, K=64 each)
  attnT   = exp(0.125 * scoresT)            (ScalarE, direct to bf16 SBUF)
  ctx_h   = v_aug_h^T @ attnT: rows 0-63 = ctxT, rows 64-127 = colsum
            (the ones columns replicate the softmax denominator 64x)
  bc      = 1/colsum  (DVE reciprocal of the replicated rows = broadcast)
  ctxT    = ctx * bc  (fused on the PSUM->SBUF copy)
  out     = ctxT^T @ Wo + bo                (natural [tok, feat], DMA out)
"""

import numpy as np
import ml_dtypes

import concourse.bass as bass
import concourse.mybir as mybir
import concourse.tile as tile
from concourse import bacc
from concourse.bass_utils import run_bass_kernel_spmd

# Problem constants (hardcoded per contest contract).
B = 16          # global batch
N = 512         # sequence length
D = 768         # embed dim
H = 12          # heads
DH = 64         # head dim
NCORES = 8
BPC = B // NCORES          # batches per core = 2
TOK = BPC * N              # tokens per core = 1024
P = 128
KC = D // P                # 6 contraction chunks
NQK = 2 * D // P           # 12 q+k m-tiles
TT = TOK // P              # 8 token tiles
HPAIRS = H // 2            # 6 head pairs

F32 = mybir.dt.float32
BF16 = mybir.dt.bfloat16
BF16_NP = ml_dtypes.bfloat16

# Module-level knobs (test.py pokes these; harness uses defaults).
TRACE = False
LAST_EXEC_NS = None
LAST_RESULTS = None

_CACHED_NC = None


def _build_nc():
    # Bacc (not raw Bass): its compile() splits sync-waits to satisfy the
    # TRN2 1-wait-per-instruction codegen constraint.
    nc = bacc.Bacc(None, target_bir_lowering=False)
    xt = nc.declare_dram_parameter("xt", [D, TOK], BF16, isOutput=False)
    wqkv = nc.declare_dram_parameter("wqkv", [D, 3 * D], BF16, isOutput=False)
    bqkv = nc.declare_dram_parameter("bqkv", [3 * D], F32, isOutput=False)
    wo = nc.declare_dram_parameter("wo", [D, D], BF16, isOutput=False)
    bo = nc.declare_dram_parameter("bo", [D], F32, isOutput=False)
    out = nc.declare_dram_parameter("out", [TOK, D], F32, isOutput=True)

    with tile.TileContext(nc) as tc:
        _body(tc, xt, wqkv, bqkv, wo, bo, out)
    nc.compile()
    return nc


def _body(tc, xt, wqkv, bqkv, wo, bo, out):
    nc = tc.nc
    AOP = mybir.AluOpType
    ACTF = mybir.ActivationFunctionType

    with (
        tc.tile_pool(name="consts", bufs=1) as consts,
        tc.tile_pool(name="work", bufs=2) as work,
        tc.tile_pool(name="psum", bufs=6, space="PSUM") as psum,
    ):
        # ---- persistent SBUF tensors -------------------------------------
        xt_sb = [consts.tile([P, TOK], BF16, tag=f"xt{k}", name=f"xt{k}") for k in range(KC)]
        wqkv_sb = [consts.tile([P, 3 * D], BF16, tag=f"wqkv{k}", name=f"wqkv{k}") for k in range(KC)]
        wo_sb = [consts.tile([P, D], BF16, tag=f"wo{k}", name=f"wo{k}") for k in range(KC)]
        bqk_sb = consts.tile([P, NQK], F32, tag="bqk")
        bv_sb = consts.tile([P, D], F32, tag="bv")
        bo_sb = consts.tile([P, D], F32, tag="bo")
        qkT = [consts.tile([P, TOK], BF16, tag=f"qkT{m}", name=f"qkT{m}") for m in range(NQK)]
        # v_aug[t]: per head h, cols 128h..128h+64 = v values, 128h+64.. = 1.0
        vaug = [consts.tile([P, H * 2 * DH], BF16, tag=f"vaug{t}", name=f"vaug{t}") for t in range(TT)]
        ctxT = [consts.tile([P, N], BF16, tag=f"ctxT{i}", name=f"ctxT{i}") for i in range(BPC * HPAIRS)]

        # ---- loads: xt on the SP ring, wqkv v-cols in parallel on the ACT
        # ring (idle this early), so the v_proj(0..3) ramp work unblocks
        # chunk-by-chunk; the 2x larger q/k columns stream in behind and are
        # consumed by the later qk_proj phase.
        # First chunk split fine-grained: the very first v_proj matmul only
        # needs xt0[:, 0:128] and wqkv0 v-cols[0:512].
        nc.sync.dma_start(out=xt_sb[0][:, 0:P], in_=xt[0:P, 0:P])
        nc.scalar.dma_start(out=wqkv_sb[0][:, 2 * D:2 * D + 512],
                            in_=wqkv[0:P, 2 * D:2 * D + 512])
        nc.sync.dma_start(out=xt_sb[0][:, P:TOK], in_=xt[0:P, P:TOK])
        nc.scalar.dma_start(out=wqkv_sb[0][:, 2 * D + 512:3 * D],
                            in_=wqkv[0:P, 2 * D + 512:3 * D])
        for k in range(1, KC):
            nc.sync.dma_start(out=xt_sb[k], in_=xt[k * P:(k + 1) * P, :])
            nc.scalar.dma_start(out=wqkv_sb[k][:, 2 * D:3 * D],
                                in_=wqkv[k * P:(k + 1) * P, 2 * D:3 * D])
        for k in range(KC):
            nc.sync.dma_start(out=wqkv_sb[k][:, 0:2 * D],
                              in_=wqkv[k * P:(k + 1) * P, 0:2 * D])
        # q/k bias, per-partition layout: bqk_sb[p, m] = bqkv[m*128 + p]
        nc.gpsimd.dma_start(
            out=bqk_sb, in_=bqkv[0:2 * D].rearrange("(m p) -> p m", p=P))
        # v / out biases broadcast along partitions
        bqkv_ap = bqkv[:]
        nc.gpsimd.dma_start(
            out=bv_sb,
            in_=bass.AP(tensor=bqkv_ap.tensor, offset=2 * D, ap=[[0, P], [1, D]]))
        bo_ap = bo[:]
        nc.gpsimd.dma_start(
            out=bo_sb,
            in_=bass.AP(tensor=bo_ap.tensor, offset=0, ap=[[0, P], [1, D]]))
        # ones columns of v_aug (persistent; written once). On the vector
        # engine so the later v_proj STT (also DVE) needs no cross-engine
        # wait for them (walrus limits STT to one sync-wait).
        for t in range(TT):
            ones_view = vaug[t].rearrange("p (h x) -> p h x", x=2 * DH)[:, :, DH:2 * DH]
            nc.vector.memset(ones_view, 1.0)
        # Pre-observe the bias DMAs on the engines that consume them, so the
        # hot-loop STT/activation ops carry only their PE wait (walrus's
        # per-instruction sync-wait budget is 1 for STT).
        scratch = consts.tile([1, 4], F32, tag="scratch")
        nc.vector.tensor_copy(out=scratch[0:1, 0:1], in_=bv_sb[0:1, 0:1])
        nc.vector.tensor_copy(out=scratch[0:1, 1:2], in_=bo_sb[0:1, 0:1])
        nc.scalar.copy(out=scratch[0:1, 2:3], in_=bqk_sb[0:1, 0:1])
        # wo on the SWDGE (gpsimd) ring: keeps the SP HWDGE ring free for the
        # xt/wqkv loads the first matmuls block on.
        for k in range(KC):
            nc.gpsimd.dma_start(out=wo_sb[k], in_=wo[k * P:(k + 1) * P, :])

        # ---- phase B0: v-projection for batch 0 token tiles --------------
        def v_proj(t):
            ps1 = psum.tile([P, 512], F32, tag="mm")
            ps2 = psum.tile([P, 256], F32, tag="mm")
            for k in range(KC):
                lhsT = xt_sb[k][:, t * P:(t + 1) * P]
                nc.tensor.matmul(ps1, lhsT, wqkv_sb[k][:, 2 * D:2 * D + 512],
                                 start=(k == 0), stop=(k == KC - 1))
                nc.tensor.matmul(ps2, lhsT, wqkv_sb[k][:, 2 * D + 512:3 * D],
                                 start=(k == 0), stop=(k == KC - 1))
            vview = vaug[t].rearrange("p (h x) -> p h x", x=2 * DH)
            bview = bv_sb.rearrange("p (h x) -> p h x", x=DH)
            nc.vector.scalar_tensor_tensor(
                out=vview[:, 0:8, 0:DH],
                in0=ps1.rearrange("p (h x) -> p h x", x=DH),
                scalar=1.0, in1=bview[:, 0:8, :],
                op0=AOP.mult, op1=AOP.add)
            nc.vector.scalar_tensor_tensor(
                out=vview[:, 8:12, 0:DH],
                in0=ps2.rearrange("p (h x) -> p h x", x=DH),
                scalar=1.0, in1=bview[:, 8:12, :],
                op0=AOP.mult, op1=AOP.add)

        # ---- phase A: q/k projection -> qkT[m] ---------------------------
        def qk_proj(hp):
            # batch-0 token halves (tch=0) of both q and k first, so the
            # first attention pair unblocks one psum-group earlier.
            for tch in range(2):
                for m in (hp, HPAIRS + hp):
                    ps = psum.tile([P, 512], F32, tag="mm")
                    for k in range(KC):
                        nc.tensor.matmul(
                            ps,
                            wqkv_sb[k][:, m * P:(m + 1) * P],
                            xt_sb[k][:, tch * 512:(tch + 1) * 512],
                            start=(k == 0), stop=(k == KC - 1))
                    nc.scalar.activation(
                        out=qkT[m][:, tch * 512:(tch + 1) * 512], in_=ps,
                        func=ACTF.Identity, bias=bqk_sb[:, m:m + 1], scale=1.0)

        # ---- phases C+D per batch ---------------------------------------
        def attention_pair(b, hp):
            ktile, qtile = qkT[HPAIRS + hp], qkT[hp]
            attn = {}
            for kc in range(4):
                for hh in range(2):
                    pr = slice(64 * hh, 64 * hh + 64)
                    ps_s = psum.tile([P, N], F32, tag="mm")
                    nc.tensor.matmul(
                        ps_s,
                        ktile[pr, b * N + kc * P: b * N + (kc + 1) * P],
                        qtile[pr, b * N:(b + 1) * N],
                        start=True, stop=True)
                    at = work.tile([P, N], BF16, tag="attn", bufs=24)
                    nc.scalar.activation(out=at, in_=ps_s, func=ACTF.Exp,
                                         scale=1.0 / np.sqrt(DH))
                    attn[(kc, hh)] = at
            for hh in range(2):
                h = 2 * hp + hh
                ps_c = psum.tile([P, N], F32, tag="ctx", bufs=2)
                for kc in range(4):
                    nc.tensor.matmul(
                        ps_c,
                        vaug[b * 4 + kc][:, 2 * DH * h: 2 * DH * (h + 1)],
                        attn[(kc, hh)],
                        start=(kc == 0), stop=(kc == 3))
                bc = work.tile([64, N], F32, tag="bc", bufs=8)
                nc.vector.reciprocal(out=bc, in_=ps_c[64:128, :])
                nc.vector.scalar_tensor_tensor(
                    out=ctxT[b * HPAIRS + hp][64 * hh:64 * hh + 64, :],
                    in0=ps_c[0:64, :], scalar=1.0, in1=bc,
                    op0=AOP.mult, op1=AOP.mult)

        def out_proj(b, tt_in_b):
            t = b * 4 + tt_in_b
            ps1 = psum.tile([P, 512], F32, tag="mm")
            ps2 = psum.tile([P, 256], F32, tag="mm")
            for hp in range(HPAIRS):
                lhsT = ctxT[b * HPAIRS + hp][:, tt_in_b * P:(tt_in_b + 1) * P]
                nc.tensor.matmul(ps1, lhsT, wo_sb[hp][:, 0:512],
                                 start=(hp == 0), stop=(hp == HPAIRS - 1))
                nc.tensor.matmul(ps2, lhsT, wo_sb[hp][:, 512:D],
                                 start=(hp == 0), stop=(hp == HPAIRS - 1))
            # bufs=8: one tile per token tile, so the STT never carries a
            # WAR wait against the previous DMA-out (STT wait budget is 1).
            o = work.tile([P, D], F32, tag="out", bufs=8)
            nc.vector.scalar_tensor_tensor(
                out=o[:, 0:512], in0=ps1, scalar=1.0, in1=bo_sb[:, 0:512],
                op0=AOP.mult, op1=AOP.add)
            nc.sync.dma_start(out=out[t * P:(t + 1) * P, 0:512], in_=o[:, 0:512])
            nc.vector.scalar_tensor_tensor(
                out=o[:, 512:D], in0=ps2, scalar=1.0, in1=bo_sb[:, 512:D],
                op0=AOP.mult, op1=AOP.add)
            nc.sync.dma_start(out=out[t * P:(t + 1) * P, 512:D], in_=o[:, 512:D])

        # Interleaved emission: v/qk projections feed attention pair-by-pair
        # so ScalarE's exp work (the attention-phase bottleneck) starts as
        # early as possible instead of serializing after all projections.
        for t in range(TT):
            v_proj(t)
        for hp in range(HPAIRS):
            qk_proj(hp)
            attention_pair(0, hp)
        for hp in range(HPAIRS):
            attention_pair(1, hp)
            if hp >= 2:
                out_proj(0, hp - 2)
        for tt_in_b in range(4):
            out_proj(1, tt_in_b)


def _get_nc():
    global _CACHED_NC
    if _CACHED_NC is None:
        _CACHED_NC = _build_nc()
    return _CACHED_NC


def kernel(x, Wqkv, bqkv, Wo, bo):
    global LAST_EXEC_NS, LAST_RESULTS
    x = np.asarray(x, dtype=np.float32)
    wqkv_bf = np.asarray(Wqkv, dtype=np.float32).astype(BF16_NP)
    wo_bf = np.asarray(Wo, dtype=np.float32).astype(BF16_NP)
    bqkv_f = np.ascontiguousarray(np.asarray(bqkv, dtype=np.float32))
    bo_f = np.ascontiguousarray(np.asarray(bo, dtype=np.float32))

    in_maps = []
    for c in range(NCORES):
        xc = x[c * BPC:(c + 1) * BPC].reshape(TOK, D).T  # [768, 1024]
        in_maps.append({
            "xt": np.ascontiguousarray(xc).astype(BF16_NP),
            "wqkv": wqkv_bf,
            "bqkv": bqkv_f,
            "wo": wo_bf,
            "bo": bo_f,
        })

    nc = _get_nc()
    res = run_bass_kernel_spmd(nc, in_maps, list(range(NCORES)), trace=TRACE)
    LAST_EXEC_NS = res.exec_time_ns
    LAST_RESULTS = res
    outs = [np.asarray(res.results[c]["out"], dtype=np.float32) for c in range(NCORES)]
    return np.concatenate(outs, axis=0).reshape(B, N, D)
